# revision 25
# baseline (speedup 1.0000x reference)
"""Trainium2 Bass kernel for routed-token transformer block (moe_routing).

Strategy (8 NeuronCores), v4:
  - Batch-parallel: core i owns sample i (B=8). NO collectives anywhere —
    the graded (cold) exec time of any kernel with a cross-core sync
    absorbs the full launch skew of the slowest core (measured ~114ms on
    a cold harness run vs <1ms warm). Instead the router k_predictor
    matvec is fully replicated: every core streams the ENTIRE kp_w1
    (256MB as fp8_e4m3) and contracts it against its own sample, so each
    core's NEFF span is independent of the others.
  - The stream is 512 x 512KB contiguous SWDGE group DMAs consumed by
    fp8 DoubleRow matmuls (x stationary [128,2,16] pairs, w moving
    [128,2,512]), accumulated in one PSUM bank chain; x itself is
    streamed in 8 sections. Pre-router block work (LN1, qkv, v_aug,
    token weights) is interleaved into the stream program order to fill
    the PE's DMA-wait gaps.
  - Activations are kept feature-major ([D, S]) end to end: LN stats via
    PE ones-matmuls + row broadcast matmuls, so there are no per-tile
    transposes. Host uploads x.T and transposes the output back.
  - Attention in fp8, query chunks of 512: per feature tile the 4 heads
    run as 2 pairs in different PE row groups so score matmuls execute
    concurrently and LDWEIGHTS overlaps. Key mask (router selection)
    enters as the per-partition bias of the exp activation; an extra
    ones-column on V yields softmax denominators. Reciprocals use the
    fast approx DVE op.
  - After each 512-query chunk, the token-local tail (wo/LN2/MLP/gated
    residual) for those tokens runs on PE/DVE underneath the next
    chunk's exp stream on ACT.
"""

import ml_dtypes
import numpy as np

import concourse.bass as bass
import concourse.bacc as bacc
import concourse.mybir as mybir
import concourse.tile as tile

F32 = mybir.dt.float32
BF16 = mybir.dt.bfloat16
F8 = mybir.dt.float8e4
AF = mybir.ActivationFunctionType
OP = mybir.AluOpType

CFG = dict(S=2048, D=256, H=8, MLP=1024, KH=512, B=8)
W8SCALE = 512.0          # host premultiplier on kp_w1 before e4m3 cast
NEG_EPS = 1e-30
NEGBIG = 1e9


def _bcast_ap(handle, p):
    """DRAM AP broadcast over p partitions (partition step 0)."""
    ap = handle.ap()
    return bass.AP(tensor=ap.tensor, offset=ap.offset,
                   ap=[[0, p]] + [list(x) for x in ap.ap[1:]])


def build_bass(cfg):
    S, D, H, MLP_, KH, B = (cfg[k] for k in ("S", "D", "H", "MLP", "KH", "B"))
    P = 128
    HD = D // H                     # 32
    NT = S // P                     # 16 key tiles
    DT = D // P                     # 2 feature tiles
    MT = MLP_ // P                  # 8 mlp feature tiles
    KT = S * D // P                 # 4096 router chunks (full kp_w1)
    G = 16                          # router chunks per group DMA (1MB fp8)
    NG = KT // G                    # 512 group DMAs
    NPAIR = G // 2                  # DoubleRow pairs per group
    TOTP = KT // 2                  # 2048 pairs total
    HPT = P // HD                   # 4 heads per feature tile
    QC = 512                        # attention query chunk
    NQC = S // QC
    TC = 512                        # token chunk for stats/mlp/epilogue
    NTC = S // TC
    VW = 40                         # padded HD+1 so ko-step (H*VW) is 16-aligned
    NSEC = 16                       # x8 stream sections
    SECC = KT // NSEC               # chunks per x section
    GSEC = NG // NSEC               # w groups per x section

    nc = bacc.Bacc()

    dp = lambda name, shape, dt=F32: nc.declare_dram_parameter(
        name, list(shape), dt, isOutput=False)

    xT_d = dp("xT", (D, S))                       # x.T fp32
    x8_d = dp("x8", (P, KT * 16), F8)             # own-sample x pairs, slot 0 of 16
    w8_d = dp("w8", (NG * P, G * KH), F8)         # FULL router w, group-contig
    amask_d = dp("amask_t", (P, NT))              # attention mask token-major
    wpw_d = dp("wpw_col", (D, 1))
    wpb_d = dp("wpb", (1, 1))
    kpb1_d = dp("kp_b1_row", (1, KH))
    kpw2_d = dp("kp_w2_row", (1, KH))
    kpb2_d = dp("kp_b2", (1, 1))
    cols6_d = dp("cols6", (D, 6))                 # ln1g ln1b ln2g ln2b bo bm2
    bqk_d = dp("bqk2", (D, 2))                    # q bias col, k bias col
    bv_d = dp("bv_row", (1, D))
    bm1_d = dp("bm1_t", (P, MT))
    wqkv_d = dp("wqkv", (D, 3 * D), BF16)
    wo_d = dp("wo", (D, D), BF16)
    wm1_d = dp("wm1", (D, MLP_), BF16)
    wm2_d = dp("wm2", (MLP_, D), BF16)
    iota_d = dp("iota_row", (4, S // 4))
    out_d = nc.declare_dram_parameter("outT", [D, S], F32, isOutput=True)

    with tile.TileContext(nc) as tc:
        with (
            tc.tile_pool(name="singles", bufs=1) as SG,
            tc.tile_pool(name="w8p", bufs=3) as W8P,
            tc.tile_pool(name="x8p", bufs=2) as X8P,
            tc.tile_pool(name="et", bufs=3) as ETP,
            tc.tile_pool(name="va", bufs=NT // 2) as VAP,
            tc.tile_pool(name="gp", bufs=MT) as GP,
            tc.tile_pool(name="tmp", bufs=4) as TMP,
            tc.tile_pool(name="rows", bufs=8) as RWS,
            tc.tile_pool(name="outp", bufs=2) as OUTP,
            tc.tile_pool(name="sm", bufs=1) as SM,
            tc.tile_pool(name="psr", bufs=1, space="PSUM") as PSR,
            tc.tile_pool(name="psc", bufs=4, space="PSUM") as PSC,
            tc.tile_pool(name="psv", bufs=1, space="PSUM") as PSV,
            tc.tile_pool(name="pst", bufs=2, space="PSUM") as PST,
        ):
            psc = lambda: PSC.tile([P, QC], F32, tag="psc", name="psc")
            pst = lambda pp=P, w=TC: PST.tile([pp, w], F32, tag="pst", name="pst")

            # ---------------- constants & weights ----------------
            ones_colf = SG.tile([P, 1], F32, tag="ones_colf")
            nc.vector.memset(ones_colf, 1.0)
            ones_colb = SG.tile([P, 1], BF16, tag="ones_colb")
            nc.vector.memset(ones_colb, 1.0)
            ones_row1 = SG.tile([1, P], F32, tag="ones_row1")
            nc.vector.memset(ones_row1, 1.0)
            ones_rowb = SG.tile([1, P], BF16, tag="ones_rowb")
            nc.vector.memset(ones_rowb, 1.0)
            ones4 = SG.tile([4, 1], F32, tag="ones4")
            nc.vector.memset(ones4, 1.0)
            eps_t = SG.tile([1, 1], F32, tag="eps")
            nc.vector.memset(eps_t, 1e-5)

            def ld(handle, shape, dt, tag, src=None):
                t = SG.tile(list(shape), dt, tag=tag, name=tag)
                nc.sync.dma_start(out=t, in_=handle.ap() if src is None else src)
                return t

            xT = [ld(xT_d, (P, S), F32, f"xT{i}",
                     src=xT_d.ap()[i * P:(i + 1) * P, :]) for i in range(DT)]
            amask_sb = ld(amask_d, (P, NT), F32, "amask")
            wpw_sb = [ld(wpw_d, (P, 1), F32, f"wpw{i}",
                         src=wpw_d.ap()[i * P:(i + 1) * P, :]) for i in range(DT)]
            wpb_sb = ld(wpb_d, (1, 1), F32, "wpb")
            wpb_col = SG.tile([P, 1], F32, tag="wpb_col")
            nc.sync.dma_start(out=wpb_col, in_=_bcast_ap(wpb_d, P))
            kpb1_sb = ld(kpb1_d, (1, KH), F32, "kpb1")
            kpw2_sb = ld(kpw2_d, (1, KH), F32, "kpw2")
            kpb2_sb = ld(kpb2_d, (1, 1), F32, "kpb2")
            cols6 = [ld(cols6_d, (P, 6), F32, f"cols6_{i}",
                        src=cols6_d.ap()[i * P:(i + 1) * P, :]) for i in range(DT)]
            bqk2 = [ld(bqk_d, (P, 2), F32, f"bqk2_{i}",
                       src=bqk_d.ap()[i * P:(i + 1) * P, :]) for i in range(DT)]
            bm1_sb = ld(bm1_d, (P, MT), F32, "bm1")
            iota_sb = ld(iota_d, (4, S // 4), F32, "iota")
            bv_b = SG.tile([P, D], F32, tag="bv_b")
            nc.sync.dma_start(out=bv_b, in_=_bcast_ap(bv_d, P))
            wq_sb = [ld(wqkv_d, (P, 3 * D), BF16, f"wqkv{i}",
                        src=wqkv_d.ap()[i * P:(i + 1) * P, :]) for i in range(DT)]
            wo_sb = [ld(wo_d, (P, D), BF16, f"wo{i}",
                        src=wo_d.ap()[i * P:(i + 1) * P, :]) for i in range(DT)]
            wm1_sb = [ld(wm1_d, (P, MLP_), BF16, f"wm1_{i}",
                         src=wm1_d.ap()[i * P:(i + 1) * P, :]) for i in range(DT)]
            wm2_sb = [ld(wm2_d, (P, D), BF16, f"wm2_{i}",
                         src=wm2_d.ap()[i * P:(i + 1) * P, :]) for i in range(MT)]

            # ---------------- pre-router block work units -------------------
            h1T = [SG.tile([P, S], BF16, tag=f"fmT{i}", bufs=2, name=f"h1T{i}")
                   for i in range(DT)]
            mrow = RWS.tile([1, S], F32, tag="srow", bufs=3, name="mrow")
            rrow = RWS.tile([1, S], F32, tag="srow", bufs=3, name="rrow")
            wrow = RWS.tile([1, S], F32, tag="srow", bufs=3, name="wrow")
            m2row = RWS.tile([1, S], F32, tag="srow", bufs=3, name="m2row")
            r2row = RWS.tile([1, S], F32, tag="srow", bufs=3, name="r2row")

            def ln_stats_unit(src_tiles, ones_src, m_out, r_out, t, w_out=None,
                              chunk_src=False):
                # per-token mean and rstd rows from feature-major tiles
                sl = slice(0, TC) if chunk_src else slice(t * TC, (t + 1) * TC)
                msl = slice(t * TC, (t + 1) * TC)
                stm = pst(1, TC)
                for i in range(DT):
                    nc.tensor.matmul(stm[0:1, 0:TC], ones_src,
                                     src_tiles[i][:, sl],
                                     start=(i == 0), stop=(i == DT - 1))
                mseg = m_out[:, msl]
                nc.vector.tensor_scalar(out=mseg, in0=stm[0:1, 0:TC],
                                        scalar1=1.0 / D, scalar2=None, op0=OP.mult)
                sts = pst(1, TC)
                for i in range(DT):
                    sq = TMP.tile([P, TC], BF16, tag="sqc", bufs=1)
                    nc.vector.tensor_mul(sq, src_tiles[i][:, sl],
                                         src_tiles[i][:, sl])
                    nc.tensor.matmul(sts[0:1, 0:TC], ones_colb, sq,
                                     start=(i == 0), stop=(i == DT - 1))
                ex2 = TMP.tile([1, TC], F32, tag="ex2", bufs=1)
                nc.vector.tensor_scalar(out=ex2, in0=sts[0:1, 0:TC],
                                        scalar1=1.0 / D, scalar2=None, op0=OP.mult)
                if w_out is not None:
                    stw = pst(1, TC)
                    for i in range(DT):
                        nc.tensor.matmul(stw[0:1, 0:TC], wpw_sb[i],
                                         src_tiles[i][:, sl],
                                         start=(i == 0), stop=(i == DT - 1))
                    nc.vector.tensor_scalar(out=w_out[:, msl],
                                            in0=stw[0:1, 0:TC],
                                            scalar1=wpb_sb[0:1, 0:1],
                                            scalar2=None, op0=OP.add)
                msq = TMP.tile([1, TC], F32, tag="msq", bufs=1)
                nc.vector.tensor_mul(msq, mseg, mseg)
                var = TMP.tile([1, TC], F32, tag="var", bufs=1)
                nc.vector.tensor_sub(var, ex2, msq)
                std = TMP.tile([1, TC], F32, tag="std", bufs=1)
                nc.scalar.activation(out=std, in_=var, func=AF.Sqrt, bias=eps_t)
                nc.vector.reciprocal_approx_fast(out=r_out[:, msl], in_=std)

            def ln_apply_unit(src_tiles, m_in, r_in, g_col, b_col, out_tiles, t,
                              out_chunk=False, chunk_src=False):
                sl = slice(0, TC) if chunk_src else slice(t * TC, (t + 1) * TC)
                msl = slice(t * TC, (t + 1) * TC)
                osl = slice(0, TC) if out_chunk else msl
                pm = pst()
                nc.tensor.matmul(pm[:, 0:TC], ones_row1, m_in[:, msl],
                                 start=True, stop=True)
                pr = pst()
                nc.tensor.matmul(pr[:, 0:TC], ones_row1, r_in[:, msl],
                                 start=True, stop=True)
                for i in range(DT):
                    t1 = TMP.tile([P, TC], F32, tag="lnt", bufs=2)
                    nc.vector.tensor_sub(t1, src_tiles[i][:, sl], pm[:, 0:TC])
                    t2 = TMP.tile([P, TC], F32, tag="lnt", bufs=2)
                    nc.vector.tensor_mul(t2, t1, pr[:, 0:TC])
                    nc.vector.tensor_scalar(out=out_tiles[i][:, osl], in0=t2,
                                            scalar1=g_col[i], scalar2=b_col[i],
                                            op0=OP.mult, op1=OP.add)

            qh = [SG.tile([P, S], F8, tag=f"qh{i}", name=f"qh{i}") for i in range(DT)]
            kh = [SG.tile([P, S], F8, tag=f"kh{i}", name=f"kh{i}") for i in range(DT)]
            qhs = [SG.tile([P, S], F8, tag=f"qhs{i}", name=f"qhs{i}")
                   for i in range(DT)]
            khs = [SG.tile([P, S], F8, tag=f"khs{i}", name=f"khs{i}")
                   for i in range(DT)]
            wtok = SM.tile([P, NT], F32, tag="wtok")
            v_aug = [VAP.tile([P, 2, H, VW], F8, tag="vaug", name="vaug")
                     for _ in range(NT // 2)]

            def qk_unit(mt, j):
                # feature tile mt of q (mt 0/1) or k (mt 2/3), token chunk j
                dst = (qh + kh)[mt]
                bcol = 0 if mt < DT else 1
                ps = pst()
                for i in range(DT):
                    nc.tensor.matmul(ps[:, 0:TC], wq_sb[i][:, mt * P:(mt + 1) * P],
                                     h1T[i][:, j * TC:(j + 1) * TC],
                                     start=(i == 0), stop=(i == DT - 1))
                nc.vector.tensor_scalar(out=dst[:, j * TC:(j + 1) * TC],
                                        in0=ps[:, 0:TC],
                                        scalar1=bqk2[mt % DT][:, bcol:bcol + 1],
                                        scalar2=None, op0=OP.add)

            def v_unit(t):
                ps = pst()
                for i in range(DT):
                    nc.tensor.matmul(ps[:, 0:D], h1T[i][:, t * P:(t + 1) * P],
                                     wq_sb[i][:, 2 * D:3 * D],
                                     start=(i == 0), stop=(i == DT - 1))
                va = v_aug[t // 2]
                par = t % 2
                nc.vector.tensor_add(va[:, par, :, 0:HD],
                                     ps[:, 0:D].rearrange("p (h d) -> p h d", h=H),
                                     bv_b.rearrange("p (h d) -> p h d", h=H))
                nc.vector.memset(va[:, par, :, HD:HD + 1], 1.0)

            def wtok_unit(tq):
                # token-major weights via PE (same fp32 contraction as row form)
                pwt = pst(P, TC)
                for t in range(tq, tq + 8):
                    for i in range(DT):
                        nc.tensor.matmul(pwt[:, (t - tq):(t - tq) + 1],
                                         xT[i][:, t * P:(t + 1) * P], wpw_sb[i],
                                         start=(i == 0), stop=(i == DT - 1))
                nc.vector.tensor_scalar(out=wtok[:, tq:tq + 8], in0=pwt[:, 0:8],
                                        scalar1=wpb_col, scalar2=None,
                                        op0=OP.add)

            units = []
            for t in range(NTC):
                units.append(lambda t=t: ln_stats_unit(xT, ones_colf, mrow, rrow,
                                                       t, w_out=wrow))
                units.append(lambda t=t: ln_apply_unit(
                    xT, mrow, rrow,
                    [cols6[i][:, 0:1] for i in range(DT)],
                    [cols6[i][:, 1:2] for i in range(DT)], h1T, t))
            for mt in range(4):
                for j in range(NTC):
                    units.append(lambda mt=mt, j=j: qk_unit(mt, j))
            for t in range(NT):
                units.append(lambda t=t: v_unit(t))
            for tq in range(0, NT, 8):
                units.append(lambda tq=tq: wtok_unit(tq))

            def shift_unit(i, half):
                # [32*k] rows of qh/kh -> rows [32*k+32 mod 128] of qhs/khs,
                # in 32-partition strips (engine span limit at offsets)
                src_t, dst_t = (qh[i], qhs[i]) if half == 0 else (kh[i], khs[i])
                for k in range(4):
                    d0 = (32 * k + 32) % 128
                    nc.vector.tensor_copy(dst_t[d0:d0 + 32, :],
                                          src_t[32 * k:32 * k + 32, :])

            for i in range(DT):
                for half in range(2):
                    units.append(lambda i=i, half=half: shift_unit(i, half))

            # ------- router stream (fp8 DoubleRow pairs, SWDGE, contiguous) --
            ps_router = PSR.tile([16, KH], F32, tag="psr", name="ps_router")
            UEVERY = max(1, NG // (len(units) + 1))

            def x8_load(s):
                t = X8P.tile([P, SECC * 16], F8, tag="x8s", name="x8s")
                nc.sync.dma_start(
                    out=t, in_=x8_d.ap()[:, s * SECC * 16:(s + 1) * SECC * 16])
                return t

            xs = x8_load(0)
            xs_next = x8_load(1)
            for g in range(NG):
                if g % GSEC == 0 and g > 0:
                    xs = xs_next
                    xs_next = x8_load(g // GSEC + 1) if g // GSEC + 1 < NSEC else None
                wt = W8P.tile([P, G * KH], F8, tag="w8c", name="w8c")
                nc.gpsimd.dma_start(out=wt, in_=w8_d.ap()[g * P:(g + 1) * P, :])
                for u in range(NPAIR):
                    pu = g * NPAIR + u
                    pl = pu - (g // GSEC) * (SECC // 2)   # pair index in section
                    lhsT = xs[:, pl * 32:(pl + 1) * 32].rearrange(
                        "p (ko b) -> p ko b", ko=2)
                    rhs = wt[:, u * 2 * KH:(u + 1) * 2 * KH].rearrange(
                        "p (ko n) -> p ko n", ko=2)
                    nc.tensor.matmul(ps_router[0:16, 0:KH], lhsT, rhs,
                                     start=(pu == 0), stop=(pu == TOTP - 1),
                                     perf_mode=mybir.MatmulPerfMode.DoubleRow)
                if g % UEVERY == UEVERY - 1 and units:
                    units.pop(0)()
            while units:
                units.pop(0)()

            # ---------------- router epilogue -> k -> sel/keymask/wsel ------
            klr = SM.tile([1, KH], F32, tag="klr")
            nc.vector.tensor_copy(klr, ps_router[0:1, 0:KH])
            # unscale W8SCALE and add kp_b1
            nc.vector.scalar_tensor_tensor(out=klr, in0=klr, scalar=1.0 / W8SCALE,
                                           in1=kpb1_sb, op0=OP.mult, op1=OP.add)
            nc.vector.scalar_tensor_tensor(out=klr, in0=klr, scalar=0.01, in1=klr,
                                           op0=OP.mult, op1=OP.max)  # leaky_relu
            scr2 = SM.tile([1, KH], F32, tag="scr2")
            kl2 = SM.tile([1, 1], F32, tag="kl2")
            nc.vector.scalar_tensor_tensor(out=scr2, in0=klr, scalar=1.0, in1=kpw2_sb,
                                           op0=OP.mult, op1=OP.mult, accum_out=kl2)
            nc.vector.tensor_add(kl2, kl2, kpb2_sb)
            sg = SM.tile([1, 1], F32, tag="sg")
            nc.scalar.activation(out=sg, in_=kl2, func=AF.Exp, scale=-1.0)
            nc.vector.tensor_scalar_add(sg, sg, 1.0)
            nc.vector.reciprocal(sg, sg)
            kv = SM.tile([1, 1], F32, tag="kv")
            nc.vector.tensor_scalar(out=kv, in0=sg, scalar1=float(S), scalar2=1.0,
                                    op0=OP.mult, op1=OP.max)
            nc.vector.tensor_scalar_min(kv, kv, float(S))
            # k = floor(clip(sig*S,1,S)) == count of j in [1,S] with j <= v
            ps4 = pst(P, TC)
            nc.tensor.matmul(ps4[0:4, 0:1], ones_row1[:, 0:4], kv, start=True, stop=True)
            vb4 = SM.tile([4, 1], F32, tag="vb4")
            nc.vector.tensor_copy(vb4, ps4[0:4, 0:1])
            kcmp = SM.tile([4, S // 4], F32, tag="kcmp", bufs=1)
            nc.vector.tensor_single_scalar(out=kcmp, in_=iota_sb, scalar=vb4, op=OP.is_le)
            cnt4 = SM.tile([4, 1], F32, tag="cnt4")
            nc.vector.tensor_reduce(out=cnt4, in_=kcmp, axis=mybir.AxisListType.X, op=OP.add)
            psk = pst(P, TC)
            nc.tensor.matmul(psk[0:1, 0:1], ones4, cnt4, start=True, stop=True)
            nc.vector.tensor_copy(kv, psk[0:1, 0:1])
            psb = pst(P, TC)
            nc.tensor.matmul(psb[:, 0:1], ones_row1, kv, start=True, stop=True)
            kb = SM.tile([P, 1], F32, tag="kb")
            nc.vector.tensor_copy(kb, psb[:, 0:1])
            # keymask first: it gates the attention exp stream
            sel01 = SM.tile([P, NT], F32, tag="sel01")
            nc.vector.tensor_single_scalar(out=sel01, in_=wtok, scalar=kb[:, 0:1], op=OP.is_gt)
            km = SM.tile([P, NT], F32, tag="km")
            nc.vector.scalar_tensor_tensor(out=km, in0=sel01, scalar=NEGBIG,
                                           in1=amask_sb, op0=OP.mult, op1=OP.add)
            nc.vector.tensor_scalar_add(km, km, -NEGBIG)
            # row-major sel/wsel for the final gating (not on the exp path)
            selr = RWS.tile([1, S], F32, tag="selr", bufs=1, name="selr")
            nc.vector.tensor_single_scalar(out=selr, in_=wrow, scalar=kb[0:1, 0:1],
                                           op=OP.is_gt)
            wselr = RWS.tile([1, S], F32, tag="wselr", bufs=1, name="wselr")
            nc.vector.tensor_mul(wselr, wrow, selr)
            # broadcast wsel to all partitions once (bf16 SBUF)
            wselB = SG.tile([P, S], BF16, tag="wselB", name="wselB")
            for t in range(NTC):
                pw = pst()
                nc.tensor.matmul(pw[:, 0:TC], ones_row1, wselr[:, t * TC:(t + 1) * TC],
                                 start=True, stop=True)
                nc.vector.tensor_copy(wselB[:, t * TC:(t + 1) * TC], pw[:, 0:TC])

            # ---------------- attention + interleaved token-local tail ------
            scale = 1.0 / float(np.sqrt(HD))
            attnT = [SG.tile([P, S], BF16, tag=f"fmT{i}", bufs=2, name=f"attnT{i}")
                     for i in range(DT)]

            def den_epilogue(h, pv, qc2):
                # denominators -> fast reciprocal -> broadcast -> scale
                ht, hr = divmod(h, HPT)
                b0 = hr * HD
                q0 = qc2 * QC
                den = TMP.tile([1, QC], F32, tag="den", bufs=1)
                nc.vector.tensor_scalar_add(den, pv[HD:HD + 1, 0:QC], NEG_EPS)
                rden = TMP.tile([1, QC], F32, tag="rden", bufs=1)
                nc.vector.reciprocal_approx_fast(out=rden, in_=den)
                denb = TMP.tile([1, QC], BF16, tag="denb", bufs=1)
                nc.vector.tensor_copy(denb, rden)
                pd = pst()
                nc.tensor.matmul(pd[0:HD, 0:QC], ones_rowb[:, 0:HD], denb,
                                 start=True, stop=True)
                av = TMP.tile([HD, QC], BF16, tag="av", bufs=1)
                nc.vector.tensor_copy(av, pv[0:HD, 0:QC])
                nc.vector.tensor_mul(
                    attnT[ht][b0:b0 + HD, q0:q0 + QC], av, pd[0:HD, 0:QC])

            def attention_chunk(qc2):
                q0 = qc2 * QC
                NP2 = NT // 2
                pending = None
                for h in range(H):
                    ht, hr = divmod(h, HPT)
                    b0 = hr * HD
                    b1 = (b0 + HD) % P
                    qT0 = qh[ht][b0:b0 + HD, :]
                    kT0 = kh[ht][b0:b0 + HD, :]
                    qT1 = qhs[ht][b1:b1 + HD, :]
                    kT1 = khs[ht][b1:b1 + HD, :]
                    pv = PSV.tile([HD + 1, QC], F32, tag="psv", name="pv",
                                  bufs=1)

                    def pv_pair(u, etp, pv=pv, h=h):
                        nc.tensor.matmul(pv[:, 0:QC],
                                         v_aug[u][:, :, h, 0:HD + 1], etp,
                                         start=(u == 0), stop=(u == NP2 - 1),
                                         perf_mode=mybir.MatmulPerfMode.DoubleRow)

                    prev_pair = None
                    etp = None
                    for skt in range(NT):
                        qT, kT, bp = ((qT0, kT0, b0) if skt % 2 == 0
                                      else (qT1, kT1, b1))
                        ps_s = psc()
                        nc.tensor.matmul(ps_s[:, 0:QC],
                                         kT[:, skt * P:(skt + 1) * P],
                                         qT[:, q0:q0 + QC],
                                         start=True, stop=True,
                                         tile_position=(bp, 0))
                        if skt % 2 == 0:
                            etp = ETP.tile([P, 2, QC], F8, tag="et", name="et",
                                           bufs=3)
                        nc.scalar.activation(out=etp[:, skt % 2, :],
                                             in_=ps_s[:, 0:QC], func=AF.Exp,
                                             scale=scale, bias=km[:, skt:skt + 1])
                        if skt == 1 and pending is not None:
                            den_epilogue(*pending)
                            pending = None
                        if skt % 2 == 1:
                            if prev_pair is not None:
                                pv_pair(*prev_pair)
                            prev_pair = (skt // 2, etp)
                    pv_pair(*prev_pair)
                    pending = (h, pv, qc2)
                den_epilogue(*pending)

            def tail_chunk(qc2):
                final = (qc2 == NQC - 1)
                mlp_ps = psc if final else pst
                t = qc2
                sl = slice(t * TC, (t + 1) * TC)
                aT = [TMP.tile([P, TC], BF16, tag=f"aTc{i}", bufs=2,
                               name=f"aTc{i}") for i in range(DT)]
                for half in range(DT):
                    ps = mlp_ps()
                    for i in range(DT):
                        nc.tensor.matmul(ps[:, 0:TC],
                                         wo_sb[i][:, half * P:(half + 1) * P],
                                         attnT[i][:, sl],
                                         start=(i == 0), stop=(i == DT - 1))
                    t1 = TMP.tile([P, TC], F32, tag="at1", bufs=2)
                    nc.vector.tensor_scalar(out=t1, in0=ps[:, 0:TC],
                                            scalar1=cols6[half][:, 4:5],
                                            scalar2=None, op0=OP.add)
                    nc.vector.tensor_add(aT[half][:, 0:TC], t1, xT[half][:, sl])
                aT_stats = [a[:, 0:TC] for a in aT]
                ln_stats_unit(aT_stats, ones_colb, m2row, r2row, t, chunk_src=True)
                h2c = [TMP.tile([P, TC], BF16, tag=f"h2c{i}", bufs=2,
                                name=f"h2c{i}") for i in range(DT)]
                ln_apply_unit(aT_stats, m2row, r2row,
                              [cols6[i][:, 2:3] for i in range(DT)],
                              [cols6[i][:, 3:4] for i in range(DT)], h2c, t,
                              out_chunk=True, chunk_src=True)
                gt = []
                for mt in range(MT):
                    ps = mlp_ps()
                    for i in range(DT):
                        nc.tensor.matmul(ps[:, 0:TC],
                                         wm1_sb[i][:, mt * P:(mt + 1) * P],
                                         h2c[i][:, 0:TC],
                                         start=(i == 0), stop=(i == DT - 1))
                    g = GP.tile([P, TC], BF16, tag="g", name="g")
                    nc.scalar.activation(out=g, in_=ps[:, 0:TC],
                                         func=AF.Gelu_apprx_tanh,
                                         bias=bm1_sb[:, mt:mt + 1])
                    gt.append(g)
                for half in range(DT):
                    pm2 = mlp_ps()
                    for mt in range(MT):
                        nc.tensor.matmul(pm2[:, 0:TC],
                                         wm2_sb[mt][:, half * P:(half + 1) * P],
                                         gt[mt], start=(mt == 0), stop=(mt == MT - 1))
                    t1 = TMP.tile([P, TC], F32, tag="ep", bufs=2)
                    nc.vector.tensor_scalar(out=t1, in0=pm2[:, 0:TC],
                                            scalar1=cols6[half][:, 5:6],
                                            scalar2=None, op0=OP.add)
                    nc.vector.tensor_add(t1, t1, aT[half][:, 0:TC])
                    t2 = TMP.tile([P, TC], F32, tag="ep", bufs=2)
                    nc.vector.tensor_mul(t2, t1, wselB[:, sl])
                    fo = OUTP.tile([P, TC], F32, tag="fo", name="fo")
                    nc.vector.tensor_add(fo, t2, xT[half][:, sl])
                    nc.sync.dma_start(out=out_d.ap()[half * P:(half + 1) * P, sl],
                                      in_=fo)

            for qc2 in range(NQC):
                attention_chunk(qc2)
                tail_chunk(qc2)

    nc.compile()
    return nc


def marshal_inputs(cfg, inputs):
    """Build per-core in_maps from full inputs (numpy)."""
    S, D, H, MLP_, KH, B = (cfg[k] for k in ("S", "D", "H", "MLP", "KH", "B"))
    P = 128
    KT = S * D // P
    G = 16
    NG = KT // G
    NT = S // P
    F8N = ml_dtypes.float8_e4m3
    BF = ml_dtypes.bfloat16

    f = lambda k: np.asarray(inputs[k], dtype=np.float32)
    x = f("x")
    amask = f("attention_mask")
    kp_w1 = np.asarray(inputs["kp_w1"])
    x_flat = x.reshape(B, S * D)

    cols6 = np.stack([f("ln1_g"), f("ln1_b"), f("ln2_g"), f("ln2_b"),
                      f("bo"), f("bm2")], axis=1)
    bqk2 = np.stack([f("bqkv")[:D], f("bqkv")[D:2 * D]], axis=1)

    # full kp_w1 as fp8, group-contiguous: same layout for every core
    w8 = (np.asarray(kp_w1, dtype=np.float32) * np.float32(W8SCALE))
    w8 = w8.astype(F8N).reshape(NG, G, P, KH).transpose(0, 2, 1, 3)
    w8 = np.ascontiguousarray(w8).reshape(NG * P, G * KH)

    shared = dict(
        w8=w8,
        wpw_col=f("wp_w").reshape(D, 1),
        wpb=f("wp_b").reshape(1, 1),
        kp_b1_row=f("kp_b1").reshape(1, KH),
        kp_w2_row=f("kp_w2").reshape(1, KH),
        kp_b2=f("kp_b2").reshape(1, 1),
        cols6=np.ascontiguousarray(cols6),
        bqk2=np.ascontiguousarray(bqk2),
        bv_row=f("bqkv")[2 * D:].reshape(1, D),
        bm1_t=np.ascontiguousarray(f("bm1").reshape(MLP_ // P, P).T),
        wqkv=f("wqkv").astype(BF),
        wo=f("wo").astype(BF),
        wm1=f("wm1").astype(BF),
        wm2=f("wm2").astype(BF),
        iota_row=np.arange(1, S + 1, dtype=np.float32).reshape(4, S // 4),
    )
    in_maps = []
    for i in range(B):
        # own-sample x pairs in slot 0 of each 16-wide block
        x8 = np.zeros((P, KT, 16), F8N)
        x8[:, :, 0] = x_flat[i].reshape(KT, P).T.astype(F8N)
        x8 = np.ascontiguousarray(x8).reshape(P, KT * 16)
        m = dict(shared)
        m["xT"] = np.ascontiguousarray(x[i].T)
        m["x8"] = x8
        m["amask_t"] = np.ascontiguousarray(amask[i, 0, 0].reshape(NT, P).T)
        in_maps.append(m)
    return in_maps


_NC_CACHE = {}


def _get_nc(cfg_key):
    if cfg_key not in _NC_CACHE:
        _NC_CACHE[cfg_key] = build_bass(CFG)
    return _NC_CACHE[cfg_key]


def run(inputs, trace=False, **kw):
    from concourse.bass_utils import run_bass_kernel_spmd

    cfg = CFG
    nc = _get_nc("full")
    in_maps = marshal_inputs(cfg, inputs)
    res = run_bass_kernel_spmd(nc, in_maps, list(range(cfg["B"])), trace=trace, **kw)
    out = np.stack([res.results[i]["outT"].T for i in range(cfg["B"])], axis=0)
    return np.ascontiguousarray(out, dtype=np.float32), res


def kernel(**inputs):
    return run(inputs)[0]
